# revision 2
# baseline (speedup 1.0000x reference)
"""Trainium2 Bass kernel for Llama-style GQA attention (B=2,S=2048,H=4096,NH=32,NKV=8,HD=128).

Sharding: tensor-parallel over heads for QKV+attention (core c owns Q-heads
4c..4c+3 and GQA KV-head c), then an AllToAll of the raw attention outputs
(4 x 1MB bf16) converts head-sharding -> token-sharding, and each core runs
the output projection for its own 512 tokens against the full Wo (streamed).
This replaces the old 8 x 8MB fp32 ReduceScatter of O-proj partials.
kernel(**inputs) takes full inputs, returns the full output.
"""

import math
import os
from contextlib import ExitStack

import numpy as np

B, S, H = 2, 2048, 4096
NH, NKV, HD = 32, 8, 128
THETA = 1000000.0
NCORES = 8
QH = NH // NCORES            # 4 q-heads per core
TOK = B * S                  # 4096 tokens (flattened batch*seq)
QO = QH * HD                 # 512 q-out dims per core
TT = TOK // 128              # 32 token tiles of 128
TS = TOK // 512              # 8 token slices of 512
SB = S // 512                # 4 q-slices of 512 per batch
KTB = S // 128               # 16 k-tiles of 128 per batch
NG = TS // 2                 # 4 A2A groups of 1024 tokens
NF = 16                      # O-proj out-dim blocks of 256
FO = H // NF                 # 256

LAST_EXEC_NS = None
LAST_RESULT = None

_compiled = {}


def _build():
    import concourse.bass as bass
    import concourse.mybir as mybir
    import concourse.tile as tile
    from concourse import bacc

    f32 = mybir.dt.float32
    f32r = mybir.dt.float32r            # fp32 w/ 11-bit mantissa: 1 PE cyc/row
    bf = mybir.dt.bfloat16
    nc = bacc.Bacc("TRN2", target_bir_lowering=False, debug=False,
                   num_devices=NCORES)

    def inp(name, shape, dt=f32):
        return nc.dram_tensor(name, shape, dt, kind="ExternalInput").ap()

    # hidden transposed and host-pre-tiled: xTt[ti] is a contiguous
    # (H, 512) block for token slice ti -> single-burst DMA tiles
    xTt = inp("xTt", (TS, H, 512), bf)
    # weight shards host-permuted to SBUF-resident layout [p, tile, out]
    wqP = inp("wqP", (128, H // 128, QO), bf)
    wkP = inp("wkP", (128, H // 128, HD), bf)
    wvP = inp("wvP", (128, H // 128, HD), bf)
    # full Wo, tiled for streaming: woF[f, p, c, o] = Wo[f*FO+o, c*128+p]
    # ([p, c, o] inner order matches the SBUF tile's flatten order)
    woF = inp("woF", (NF, 128, NH, FO), bf)
    bqP = inp("bqP", (128, QH))         # bq shard as [d, head]
    bkP = inp("bkP", (128, 1))
    bvP = inp("bvP", (128, 1))
    boB = inp("boB", (1, H), bf)        # full bo
    cosT = inp("cosT", (HD, TOK), bf)
    sinT = inp("sinT", (HD, TOK), bf)
    rotM = inp("rotM", (HD, HD), bf)  # lhsT for rotate_half_interleaved
    ident = inp("ident", (128, 128), f32r)
    ones = inp("ones", (128, 128), bf)
    mask128 = inp("mask128", (128, 128), bf)  # strict-diagonal causal triangle

    out = nc.dram_tensor("out", (TOK // NCORES, H), f32, kind="ExternalOutput").ap()
    # A2A buffers: one pair per 1024-token group so each collective's
    # dependencies stay scoped to its group (comms overlap compute).
    # Layout [dest core j, d partition, head, token] so SBUF<->DRAM DMAs are
    # contiguous 1KB lines per partition.
    a2a_ins = [nc.dram_tensor(f"a2a_in{g}", (NCORES, 128, QH, 128), bf,
                              kind="Internal").ap() for g in range(NG)]
    a2a_outs = [nc.dram_tensor(f"a2a_out{g}", (NCORES, 128, QH, 128), bf,
                               kind="Internal").ap() for g in range(NG)]

    inv_sqrt_hd = 1.0 / math.sqrt(HD)

    def mm(out, lhsT, rhs, **kw):
        nc.tensor.matmul(out, lhsT, rhs, **kw)

    with tile.TileContext(nc) as tc, ExitStack() as stk:
        # ---------------- constants + persistent activations ----------------
        cpool = stk.enter_context(tc.tile_pool(name="consts", bufs=1))
        apool = stk.enter_context(tc.tile_pool(name="acts", bufs=1))

        cos_sb = cpool.tile([128, TOK], bf)
        nc.sync.dma_start(cos_sb[:], cosT[:])
        sin_sb = cpool.tile([128, TOK], bf)
        nc.sync.dma_start(sin_sb[:], sinT[:])
        rot_sb = cpool.tile([128, 128], bf)
        nc.sync.dma_start(rot_sb[:], rotM[:])
        id_sb = cpool.tile([128, 128], f32r)
        nc.sync.dma_start(id_sb[:], ident[:])
        ones_sb = cpool.tile([128, 128], bf)
        nc.sync.dma_start(ones_sb[:], ones[:])
        bq_sb = cpool.tile([128, QH], f32)
        nc.sync.dma_start(bq_sb[:], bqP[:])
        bk_sb = cpool.tile([128, 1], f32)
        nc.sync.dma_start(bk_sb[:], bkP[:])
        bv_sb = cpool.tile([128, 1], f32)
        nc.sync.dma_start(bv_sb[:], bvP[:])
        bo_sb = cpool.tile([1, H], bf)
        nc.sync.dma_start(bo_sb[:], boB[:])
        # bo broadcast to all partitions, used in the O-proj PSUM drain
        bo_bc = cpool.tile([128, H], bf)
        nc.gpsimd.partition_broadcast(bo_bc[:], bo_sb[:])

        mask_sb = cpool.tile([128, 128], bf)
        nc.sync.dma_start(mask_sb[:], mask128[:])

        # resident QKV weight shards: [d-in partition, h tile, out]
        wq_res = apool.tile([128, H // 128, QO], bf)
        nc.sync.dma_start(wq_res[:], wqP[:])
        wk_res = apool.tile([128, H // 128, HD], bf)
        nc.sync.dma_start(wk_res[:], wkP[:])
        wv_res = apool.tile([128, H // 128, HD], bf)
        nc.sync.dma_start(wv_res[:], wvP[:])

        KT = apool.tile([128, TOK], bf)        # K^T (rope'd), grows causally
        Vsb = apool.tile([128, TT, 128], bf)   # V in [t mod 128, t tile, d]
        # post-A2A attention outputs: [d partition, group, src core, head, tok]
        oin = apool.tile([128, NG, NCORES, QH, 128], bf)

        sp = stk.enter_context(tc.tile_pool(name="streams", bufs=4))
        tp = stk.enter_context(tc.tile_pool(name="tmps", bufs=2))
        qtp = stk.enter_context(tc.tile_pool(name="qts", bufs=2))
        vtp = stk.enter_context(tc.tile_pool(name="vts", bufs=2))
        atp = stk.enter_context(tc.tile_pool(name="attw", bufs=4))
        smp = stk.enter_context(tc.tile_pool(name="smalls", bufs=2))
        anp = stk.enter_context(tc.tile_pool(name="atn", bufs=2))
        wop = stk.enter_context(tc.tile_pool(name="wo", bufs=2))
        stp = stk.enter_context(tc.tile_pool(name="ostage", bufs=2))
        pp = stk.enter_context(tc.tile_pool(name="ps", bufs=8, space="PSUM"))

        def ps_tile(shape=(128, 512)):
            return pp.tile(list(shape), f32, name="ps", tag="ps")

        for ti in range(TS):
            b, j = ti // SB, ti % SB
            t0 = ti * 512
            # ---- QKV projection for this token slice (accumulate over h) ----
            psq = [ps_tile() for _ in range(QH)]
            psk = ps_tile()
            psv = ps_tile()
            for hi in range(H // 128):
                h0 = hi * 128
                xt = sp.tile([128, 512], bf, name="xt")
                nc.sync.dma_start(xt[:], xTt[ti, h0:h0 + 128, :])
                st = (hi == 0)
                en = (hi == H // 128 - 1)
                for q in range(QH):
                    mm(psq[q][:], wq_res[:, hi, q * 128:(q + 1) * 128],
                       xt[:], start=st, stop=en)
                mm(psk[:], wk_res[:, hi, :], xt[:], start=st, stop=en)
                mm(psv[:], wv_res[:, hi, :], xt[:], start=st, stop=en)

            # bias add (per-partition) while draining PSUM
            QTs = qtp.tile([128, QH, 512], bf, name="QTs")
            VTs = vtp.tile([128, 512], f32r, name="VTs")
            for q in range(QH):
                nc.scalar.add(QTs[:, q, :], psq[q][:], bq_sb[:, q:q + 1])
            nc.scalar.add(KT[:, t0:t0 + 512], psk[:], bk_sb[:, 0:1])
            nc.scalar.add(VTs[:], psv[:], bv_sb[:, 0:1])

            # rope in place on QT / KT slices
            def rope(ap_slice):
                rps = ps_tile()
                mm(rps[:], rot_sb[:], ap_slice, start=True, stop=True)
                t1 = tp.tile([128, 512], f32, name="t1")
                nc.vector.tensor_mul(t1[:], ap_slice, cos_sb[:, t0:t0 + 512])
                t2 = tp.tile([128, 512], f32, name="t2")
                nc.vector.tensor_mul(t2[:], rps[:], sin_sb[:, t0:t0 + 512])
                nc.vector.tensor_add(ap_slice, t1[:], t2[:])

            rope(KT[:, t0:t0 + 512])   # first: scores need K before all Q heads
            for q in range(QH):
                rope(QTs[:, q, :])

            # V^T -> V (PE transpose of 128x128 blocks)
            for s4 in range(4):
                g4 = ti * 4 + s4
                vps = pp.tile([128, 128], f32r, name="vps", tag="ps")
                nc.tensor.transpose(vps[:], VTs[:, s4 * 128:(s4 + 1) * 128],
                                    id_sb[:])
                nc.scalar.copy(Vsb[:, g4, :], vps[:])

            # ---- causal attention for this q slice ----
            # Diagonal k-tiles (ki = 4j + a) only attend to q >= 128a: shrink
            # the matmul free range instead of masking the whole tile; only
            # the strict-diagonal 128-col strip needs the triangle mask.
            nk = 4 * j + 4                # k tiles of 128 within batch b
            ATn = anp.tile([128, QH, 512], bf, name="ATn")
            for h in range(QH):
                dn_ps = ps_tile((1, 512))
                at_ps = ps_tile()
                for ki in range(nk):
                    kg = b * KTB + ki
                    a = ki - 4 * j
                    off = 128 * a if a >= 0 else 0
                    sc_ps = ps_tile()
                    mm(sc_ps[:, off:], KT[:, kg * 128:(kg + 1) * 128],
                       QTs[:, h, off:], start=True, stop=True)
                    a_sb = atp.tile([128, 512], bf, name="a_sb")
                    nc.scalar.activation(a_sb[:, off:], sc_ps[:, off:],
                                         mybir.ActivationFunctionType.Exp,
                                         scale=inv_sqrt_hd)
                    if a >= 0:
                        nc.vector.tensor_mul(a_sb[:, off:off + 128],
                                             a_sb[:, off:off + 128],
                                             mask_sb[:])
                    mm(dn_ps[:, off:], ones_sb[:, 0:1], a_sb[:, off:],
                       start=(ki == 0), stop=(ki == nk - 1))
                    mm(at_ps[:, off:], Vsb[:, kg, :], a_sb[:, off:],
                       start=(ki == 0), stop=(ki == nk - 1))
                dr = smp.tile([1, 512], f32, name="dr")
                nc.vector.reciprocal(dr[:], dn_ps[:])
                rb = smp.tile([128, 512], f32, name="rb")
                nc.gpsimd.partition_broadcast(rb[:], dr[:])
                nc.vector.tensor_mul(ATn[:, h, :], at_ps[:], rb[:])

            # ---- stage this slice's attention output for the A2A ----
            g, half = ti // 2, ti % 2
            for jj in range(4):
                nc.sync.dma_start(
                    a2a_ins[g][4 * half + jj, :, :, :],
                    ATn[:, :, jj * 128:(jj + 1) * 128])
            if half == 1:
                # group complete: exchange head-shards for token-shards
                nc.gpsimd.collective_compute(
                    "AllToAll", mybir.AluOpType.bypass,
                    replica_groups=[list(range(NCORES))],
                    ins=[a2a_ins[g].opt()], outs=[a2a_outs[g].opt()],
                )
                for s in range(NCORES):
                    nc.sync.dma_start(oin[:, g, s, :, :],
                                      a2a_outs[g][s, :, :, :])

        # ---- O-projection for this core's 512 tokens, full Wo streamed ----
        for f in range(NF):
            f0 = f * FO
            wo_sb = wop.tile([128, NH, FO], bf, name="wo_sb")
            nc.sync.dma_start(wo_sb[:], woF[f, :, :, :])
            for g in range(NG):
                op_ps = ps_tile((128, FO))
                for s in range(NCORES):
                    for hh in range(QH):
                        c = s * QH + hh
                        mm(op_ps[:], oin[:, g, s, hh, :], wo_sb[:, c, :],
                           start=(c == 0), stop=(c == NH - 1))
                st_t = stp.tile([128, FO], f32, name="st_t")
                nc.vector.tensor_add(st_t[:], op_ps[:], bo_bc[:, f0:f0 + FO])
                nc.sync.dma_start(out[g * 128:(g + 1) * 128, f0:f0 + FO],
                                  st_t[:])

    nc.compile()
    return nc


def _host_inputs(hidden_states, position_ids, Wq, bq, Wk, bk, Wv, bv, Wo, bo):
    import ml_dtypes
    bf16 = ml_dtypes.bfloat16
    f = np.float32
    X = np.asarray(hidden_states, f).reshape(TOK, H)
    xT = np.ascontiguousarray(X.T).astype(bf16)
    xTt = np.ascontiguousarray(xT.reshape(H, TS, 512).transpose(1, 0, 2))

    pos = np.asarray(position_ids).astype(f).reshape(TOK)
    inv_freq = (1.0 / (THETA ** (np.arange(0, HD, 2, dtype=f) / HD))).astype(f)
    M = inv_freq[:, None] * pos[None, :]              # [64, TOK]
    cosT = np.repeat(np.cos(M), 2, axis=0).astype(f)  # [128, TOK]
    sinT = np.repeat(np.sin(M), 2, axis=0).astype(f)

    rotM = np.zeros((HD, HD), f)
    for i in range(HD // 2):
        rotM[2 * i + 1, 2 * i] = -1.0   # out[2i]   = -in[2i+1]
        rotM[2 * i, 2 * i + 1] = 1.0    # out[2i+1] =  in[2i]

    Wq, Wk, Wv, Wo = (np.asarray(a, f) for a in (Wq, Wk, Wv, Wo))
    bq, bk, bv, bo = (np.asarray(a, f) for a in (bq, bk, bv, bo))

    # full Wo tiled for streaming: woF[f, p, c, o] = Wo[f*FO+o, c*128+p]
    woF = np.ascontiguousarray(
        Wo.T.reshape(NH, 128, NF, FO).transpose(2, 1, 0, 3)).astype(bf16)

    shared = {
        "xTt": xTt, "cosT": cosT.astype(bf16), "sinT": sinT.astype(bf16),
        "rotM": rotM.astype(bf16),
        "ident": np.eye(128, dtype=f), "ones": np.ones((128, 128), bf16),
        "woF": woF,
        "boB": np.asarray(bo, f).reshape(1, H).astype(bf16),
        "mask128": (np.arange(128)[None, :]
                    - np.arange(128)[:, None] >= 0).astype(bf16),
    }
    in_maps = []
    for c in range(NCORES):
        m = dict(shared)
        # [p, h-tile, o] resident layout: wT[h, o] with h = ht*128 + p
        wqT = Wq[c * QO:(c + 1) * QO, :].T.reshape(H // 128, 128, QO)
        m["wqP"] = np.ascontiguousarray(wqT.transpose(1, 0, 2)).astype(bf16)
        wkT = Wk[c * HD:(c + 1) * HD, :].T.reshape(H // 128, 128, HD)
        m["wkP"] = np.ascontiguousarray(wkT.transpose(1, 0, 2)).astype(bf16)
        wvT = Wv[c * HD:(c + 1) * HD, :].T.reshape(H // 128, 128, HD)
        m["wvP"] = np.ascontiguousarray(wvT.transpose(1, 0, 2)).astype(bf16)
        m["bqP"] = np.ascontiguousarray(bq[c * QO:(c + 1) * QO].reshape(QH, 128).T)
        m["bkP"] = bk[c * HD:(c + 1) * HD].reshape(128, 1).copy()
        m["bvP"] = bv[c * HD:(c + 1) * HD].reshape(128, 1).copy()
        in_maps.append(m)
    return in_maps


def _gather(results):
    # core c's out row (g*128 + r) holds global token g*1024 + c*128 + r
    stacked = np.stack([results[c]["out"] for c in range(NCORES)])
    full = stacked.reshape(NCORES, NG, 128, H).transpose(1, 0, 2, 3)
    return np.ascontiguousarray(full).reshape(B, S, H)


def kernel(hidden_states, position_ids, Wq, bq, Wk, bk, Wv, bv, Wo, bo):
    global LAST_EXEC_NS, LAST_RESULT
    from concourse.bass_utils import run_bass_kernel_spmd

    if "nc" not in _compiled:
        _compiled["nc"] = _build()
    nc = _compiled["nc"]

    in_maps = _host_inputs(hidden_states, position_ids,
                           Wq, bq, Wk, bk, Wv, bv, Wo, bo)
    trace = os.environ.get("KERNEL_TRACE", "0") == "1"
    res = run_bass_kernel_spmd(nc, in_maps, core_ids=list(range(NCORES)),
                               trace=trace)
    LAST_EXEC_NS = res.exec_time_ns
    LAST_RESULT = res
    return _gather(res.results)


# revision 4
# speedup vs baseline: 1.0350x; 1.0350x over previous
"""Trainium2 Bass kernel for Llama-style GQA attention (B=2,S=2048,H=4096,NH=32,NKV=8,HD=128).

Sharding: tensor-parallel over heads for QKV+attention (core c owns Q-heads
4c..4c+3 and GQA KV-head c), then an AllToAll of the raw attention outputs
(4 x 1MB bf16) converts head-sharding -> token-sharding, and each core runs
the output projection for its own 512 tokens against the full Wo (streamed).
This replaces the old 8 x 8MB fp32 ReduceScatter of O-proj partials.
kernel(**inputs) takes full inputs, returns the full output.
"""

import math
import os
from contextlib import ExitStack

import numpy as np

B, S, H = 2, 2048, 4096
NH, NKV, HD = 32, 8, 128
THETA = 1000000.0
NCORES = 8
QH = NH // NCORES            # 4 q-heads per core
TOK = B * S                  # 4096 tokens (flattened batch*seq)
QO = QH * HD                 # 512 q-out dims per core
TT = TOK // 128              # 32 token tiles of 128
TS = TOK // 512              # 8 token slices of 512
SB = S // 512                # 4 q-slices of 512 per batch
KTB = S // 128               # 16 k-tiles of 128 per batch
NG = TS // 2                 # 4 A2A groups of 1024 tokens
NF = 16                      # O-proj out-dim blocks of 256
FO = H // NF                 # 256

LAST_EXEC_NS = None
LAST_RESULT = None

_compiled = {}


def _build():
    import concourse.bass as bass
    import concourse.mybir as mybir
    import concourse.tile as tile
    from concourse import bacc

    f32 = mybir.dt.float32
    f32r = mybir.dt.float32r            # fp32 w/ 11-bit mantissa: 1 PE cyc/row
    bf = mybir.dt.bfloat16
    nc = bacc.Bacc("TRN2", target_bir_lowering=False, debug=False,
                   num_devices=NCORES)

    def inp(name, shape, dt=f32):
        return nc.dram_tensor(name, shape, dt, kind="ExternalInput").ap()

    # hidden transposed and host-pre-tiled: xTt[ti] is a contiguous
    # (H, 512) block for token slice ti -> single-burst DMA tiles
    xTt = inp("xTt", (TS, H, 512), bf)
    # weight shards host-permuted to SBUF-resident layout [p, tile, out]
    wqP = inp("wqP", (128, H // 128, QO), bf)
    wkP = inp("wkP", (128, H // 128, HD), bf)
    wvP = inp("wvP", (128, H // 128, HD), bf)
    # full Wo, tiled for streaming: woF[f, p, c, o] = Wo[f*FO+o, c*128+p]
    # ([p, c, o] inner order matches the SBUF tile's flatten order)
    woF = inp("woF", (NF, 128, NH, FO), bf)
    bqP = inp("bqP", (128, QH))         # bq shard as [d, head]
    bkP = inp("bkP", (128, 1))
    bvP = inp("bvP", (128, 1))
    boB = inp("boB", (1, H), bf)        # full bo
    cosT = inp("cosT", (HD, TOK), bf)
    sinT = inp("sinT", (HD, TOK), bf)
    rotM = inp("rotM", (HD, HD), bf)  # lhsT for rotate_half_interleaved
    ident = inp("ident", (128, 128), f32r)
    onesr = inp("onesr", (128, 1), f32r)
    ones = inp("ones", (128, 128), bf)
    mask128 = inp("mask128", (128, 128), bf)  # strict-diagonal causal triangle

    out = nc.dram_tensor("out", (TOK // NCORES, H), f32, kind="ExternalOutput").ap()
    # A2A buffers: one pair per 1024-token group so each collective's
    # dependencies stay scoped to its group (comms overlap compute).
    # Layout [dest core j, d partition, head, token] so SBUF<->DRAM DMAs are
    # contiguous 1KB lines per partition.
    a2a_ins = [nc.dram_tensor(f"a2a_in{g}", (NCORES, 128, QH, 128), bf,
                              kind="Internal").ap() for g in range(NG)]
    a2a_outs = [nc.dram_tensor(f"a2a_out{g}", (NCORES, 128, QH, 128), bf,
                               kind="Internal").ap() for g in range(NG)]

    inv_sqrt_hd = 1.0 / math.sqrt(HD)

    def mm(out, lhsT, rhs, **kw):
        nc.tensor.matmul(out, lhsT, rhs, **kw)

    with tile.TileContext(nc) as tc, ExitStack() as stk:
        # ---------------- constants + persistent activations ----------------
        cpool = stk.enter_context(tc.tile_pool(name="consts", bufs=1))
        apool = stk.enter_context(tc.tile_pool(name="acts", bufs=1))

        cos_sb = cpool.tile([128, TOK], bf)
        nc.scalar.dma_start(cos_sb[:], cosT[:])
        sin_sb = cpool.tile([128, TOK], bf)
        nc.scalar.dma_start(sin_sb[:], sinT[:])
        rot_sb = cpool.tile([128, 128], bf)
        nc.scalar.dma_start(rot_sb[:], rotM[:])
        id_sb = cpool.tile([128, 128], f32r)
        nc.scalar.dma_start(id_sb[:], ident[:])
        onesr_sb = cpool.tile([128, 1], f32r)
        nc.scalar.dma_start(onesr_sb[:], onesr[:])
        ones_sb = cpool.tile([128, 128], bf)
        nc.scalar.dma_start(ones_sb[:], ones[:])
        bq_sb = cpool.tile([128, QH], f32)
        nc.scalar.dma_start(bq_sb[:], bqP[:])
        bk_sb = cpool.tile([128, 1], f32)
        nc.scalar.dma_start(bk_sb[:], bkP[:])
        bv_sb = cpool.tile([128, 1], f32)
        nc.scalar.dma_start(bv_sb[:], bvP[:])
        bo_sb = cpool.tile([1, H], bf)
        nc.scalar.dma_start(bo_sb[:], boB[:])
        # bo broadcast to all partitions, used in the O-proj PSUM drain
        bo_bc = cpool.tile([128, H], bf)
        nc.gpsimd.partition_broadcast(bo_bc[:], bo_sb[:])

        mask_sb = cpool.tile([128, 128], bf)
        nc.scalar.dma_start(mask_sb[:], mask128[:])

        # resident QKV weight shards: [d-in partition, h tile, out]
        wq_res = apool.tile([128, H // 128, QO], bf)
        nc.sync.dma_start(wq_res[:], wqP[:])
        wk_res = apool.tile([128, H // 128, HD], bf)
        nc.sync.dma_start(wk_res[:], wkP[:])
        wv_res = apool.tile([128, H // 128, HD], bf)
        nc.sync.dma_start(wv_res[:], wvP[:])

        KT = apool.tile([128, TOK], bf)        # K^T (rope'd), grows causally
        Vsb = apool.tile([128, TT, 128], bf)   # V in [t mod 128, t tile, d]
        # post-A2A attention outputs: [d partition, group, src core, head, tok]
        oin = apool.tile([128, NG, NCORES, QH, 128], bf)

        sp = stk.enter_context(tc.tile_pool(name="streams", bufs=4))
        tp = stk.enter_context(tc.tile_pool(name="tmps", bufs=2))
        acp = stk.enter_context(tc.tile_pool(name="accs", bufs=2))
        qtp = stk.enter_context(tc.tile_pool(name="qts", bufs=2))
        vtp = stk.enter_context(tc.tile_pool(name="vts", bufs=2))
        atp = stk.enter_context(tc.tile_pool(name="attw", bufs=4))
        smp = stk.enter_context(tc.tile_pool(name="smalls", bufs=2))
        anp = stk.enter_context(tc.tile_pool(name="atn", bufs=2))
        wop = stk.enter_context(tc.tile_pool(name="wo", bufs=2))
        stp = stk.enter_context(tc.tile_pool(name="ostage", bufs=2))
        pp = stk.enter_context(tc.tile_pool(name="ps", bufs=8, space="PSUM"))

        def ps_tile(shape=(128, 512)):
            return pp.tile(list(shape), f32, name="ps", tag="ps")

        # prefetch the first O-proj weight block during the slice loop
        # (scalar HWDGE queue: keeps the sync queue free for wq/wk/wv + xt)
        wo_first = wop.tile([128, NH, FO], bf, name="wo_sb")
        nc.scalar.dma_start(wo_first[:], woF[0, :, :, :])

        for ti in range(TS):
            b, j = ti // SB, ti % SB
            t0 = ti * 512
            # ---- QKV projection for this token slice (accumulate over h) ----
            psq = [ps_tile() for _ in range(QH)]
            psk = ps_tile()
            psv = ps_tile()
            for hi in range(H // 128):
                h0 = hi * 128
                xt = sp.tile([128, 512], bf, name="xt")
                nc.sync.dma_start(xt[:], xTt[ti, h0:h0 + 128, :])
                st = (hi == 0)
                en = (hi == H // 128 - 1)
                for q in range(QH):
                    mm(psq[q][:], wq_res[:, hi, q * 128:(q + 1) * 128],
                       xt[:], start=st, stop=en)
                mm(psk[:], wk_res[:, hi, :], xt[:], start=st, stop=en)
                mm(psv[:], wv_res[:, hi, :], xt[:], start=st, stop=en)

            # bias add (per-partition) while draining PSUM
            QTs = qtp.tile([128, QH, 512], bf, name="QTs")
            VTs = vtp.tile([128, 512], f32r, name="VTs")
            for q in range(QH):
                nc.scalar.add(QTs[:, q, :], psq[q][:], bq_sb[:, q:q + 1])
            nc.scalar.add(KT[:, t0:t0 + 512], psk[:], bk_sb[:, 0:1])
            nc.scalar.add(VTs[:], psv[:], bv_sb[:, 0:1])

            # rope in place on QT / KT slices
            def rope(ap_slice):
                rps = ps_tile()
                mm(rps[:], rot_sb[:], ap_slice, start=True, stop=True)
                t1 = tp.tile([128, 512], bf, name="t1")
                nc.vector.tensor_mul(t1[:], ap_slice, cos_sb[:, t0:t0 + 512])
                t2 = tp.tile([128, 512], bf, name="t2")
                nc.vector.tensor_mul(t2[:], rps[:], sin_sb[:, t0:t0 + 512])
                nc.vector.tensor_add(ap_slice, t1[:], t2[:])

            rope(KT[:, t0:t0 + 512])   # first: scores need K before all Q heads
            for q in range(QH):
                rope(QTs[:, q, :])

            # V^T -> V (PE transpose of 128x128 blocks)
            for s4 in range(4):
                g4 = ti * 4 + s4
                vps = pp.tile([128, 128], f32r, name="vps", tag="ps")
                nc.tensor.transpose(vps[:], VTs[:, s4 * 128:(s4 + 1) * 128],
                                    id_sb[:])
                nc.scalar.copy(Vsb[:, g4, :], vps[:])

            # ---- causal attention for this q slice ----
            # Diagonal k-tiles (ki = 4j + a) only attend to q >= 128a: shrink
            # the matmul free range instead of masking the whole tile; only
            # the strict-diagonal 128-col strip needs the triangle mask.
            nk = 4 * j + 4                # k tiles of 128 within batch b
            ATn = anp.tile([128, QH, 512], bf, name="ATn")
            for h in range(QH):
                at_ps = ps_tile()
                # exp-sums accumulate on DVE (acc), PE does one final
                # ones-matmul per head instead of one per k-tile
                acc = acp.tile([128, 512], f32r, name="acc")
                a_sbs = {}

                def emit_sc(ki):
                    kg = b * KTB + ki
                    a = ki - 4 * j
                    off = 128 * a if a >= 0 else 0
                    sc_ps = ps_tile()
                    mm(sc_ps[:, off:], KT[:, kg * 128:(kg + 1) * 128],
                       QTs[:, h, off:], start=True, stop=True)
                    a_sb = atp.tile([128, 512], bf, name="a_sb")
                    nc.scalar.activation(a_sb[:, off:], sc_ps[:, off:],
                                         mybir.ActivationFunctionType.Exp,
                                         scale=inv_sqrt_hd)
                    if a >= 0:
                        nc.vector.tensor_mul(a_sb[:, off:off + 128],
                                             a_sb[:, off:off + 128],
                                             mask_sb[:])
                    a_sbs[ki] = (a_sb, off)

                # 2-deep software pipeline: scores/exp run ahead of the AV
                # matmuls so the PE never waits on the ACT/DVE chain
                emit_sc(0)
                if nk > 1:
                    emit_sc(1)
                for ki in range(nk):
                    if ki + 2 < nk:
                        emit_sc(ki + 2)
                    a_sb, off = a_sbs.pop(ki)
                    kg = b * KTB + ki
                    mm(at_ps[:, off:], Vsb[:, kg, :], a_sb[:, off:],
                       start=(ki == 0), stop=(ki == nk - 1))
                    if ki == 0:
                        nc.vector.tensor_scalar_add(acc[:], a_sb[:], 0.0)
                    else:
                        nc.vector.tensor_add(acc[:, off:], acc[:, off:],
                                             a_sb[:, off:])
                dn_ps = ps_tile((1, 512))
                mm(dn_ps[:], onesr_sb[:, 0:1], acc[:], start=True, stop=True)
                dr = smp.tile([1, 512], f32, name="dr")
                nc.vector.reciprocal(dr[:], dn_ps[:])
                rb = smp.tile([128, 512], f32, name="rb")
                nc.gpsimd.partition_broadcast(rb[:], dr[:])
                nc.vector.tensor_mul(ATn[:, h, :], at_ps[:], rb[:])

            # ---- stage this slice's attention output for the A2A ----
            g, half = ti // 2, ti % 2
            for jj in range(4):
                nc.scalar.dma_start(
                    a2a_ins[g][4 * half + jj, :, :, :],
                    ATn[:, :, jj * 128:(jj + 1) * 128])
            if half == 1:
                # group complete: exchange head-shards for token-shards
                nc.gpsimd.collective_compute(
                    "AllToAll", mybir.AluOpType.bypass,
                    replica_groups=[list(range(NCORES))],
                    ins=[a2a_ins[g].opt()], outs=[a2a_outs[g].opt()],
                )
                # scalar HWDGE queue: these wait on the collective, and on
                # the sync queue they would block later slices' xt loads
                for s in range(NCORES):
                    nc.scalar.dma_start(oin[:, g, s, :, :],
                                        a2a_outs[g][s, :, :, :])

        # ---- O-projection for this core's 512 tokens, full Wo streamed ----
        for f in range(NF):
            f0 = f * FO
            if f == 0:
                wo_sb = wo_first
            else:
                wo_sb = wop.tile([128, NH, FO], bf, name="wo_sb")
                nc.sync.dma_start(wo_sb[:], woF[f, :, :, :])
            for g in range(NG):
                op_ps = ps_tile((128, FO))
                for s in range(NCORES):
                    for hh in range(QH):
                        c = s * QH + hh
                        mm(op_ps[:], oin[:, g, s, hh, :], wo_sb[:, c, :],
                           start=(c == 0), stop=(c == NH - 1))
                st_t = stp.tile([128, FO], f32, name="st_t")
                nc.vector.tensor_add(st_t[:], op_ps[:], bo_bc[:, f0:f0 + FO])
                nc.sync.dma_start(out[g * 128:(g + 1) * 128, f0:f0 + FO],
                                  st_t[:])

    nc.compile()
    return nc


def _host_inputs(hidden_states, position_ids, Wq, bq, Wk, bk, Wv, bv, Wo, bo):
    import ml_dtypes
    bf16 = ml_dtypes.bfloat16
    f = np.float32
    X = np.asarray(hidden_states, f).reshape(TOK, H)
    xT = np.ascontiguousarray(X.T).astype(bf16)
    xTt = np.ascontiguousarray(xT.reshape(H, TS, 512).transpose(1, 0, 2))

    pos = np.asarray(position_ids).astype(f).reshape(TOK)
    inv_freq = (1.0 / (THETA ** (np.arange(0, HD, 2, dtype=f) / HD))).astype(f)
    M = inv_freq[:, None] * pos[None, :]              # [64, TOK]
    cosT = np.repeat(np.cos(M), 2, axis=0).astype(f)  # [128, TOK]
    sinT = np.repeat(np.sin(M), 2, axis=0).astype(f)

    rotM = np.zeros((HD, HD), f)
    for i in range(HD // 2):
        rotM[2 * i + 1, 2 * i] = -1.0   # out[2i]   = -in[2i+1]
        rotM[2 * i, 2 * i + 1] = 1.0    # out[2i+1] =  in[2i]

    Wq, Wk, Wv, Wo = (np.asarray(a, f) for a in (Wq, Wk, Wv, Wo))
    bq, bk, bv, bo = (np.asarray(a, f) for a in (bq, bk, bv, bo))

    # full Wo tiled for streaming: woF[f, p, c, o] = Wo[f*FO+o, c*128+p]
    woF = np.ascontiguousarray(
        Wo.T.reshape(NH, 128, NF, FO).transpose(2, 1, 0, 3)).astype(bf16)

    shared = {
        "xTt": xTt, "cosT": cosT.astype(bf16), "sinT": sinT.astype(bf16),
        "rotM": rotM.astype(bf16),
        "ident": np.eye(128, dtype=f), "ones": np.ones((128, 128), bf16),
        "onesr": np.ones((128, 1), f),
        "woF": woF,
        "boB": np.asarray(bo, f).reshape(1, H).astype(bf16),
        "mask128": (np.arange(128)[None, :]
                    - np.arange(128)[:, None] >= 0).astype(bf16),
    }
    in_maps = []
    for c in range(NCORES):
        m = dict(shared)
        # [p, h-tile, o] resident layout: wT[h, o] with h = ht*128 + p
        wqT = Wq[c * QO:(c + 1) * QO, :].T.reshape(H // 128, 128, QO)
        m["wqP"] = np.ascontiguousarray(wqT.transpose(1, 0, 2)).astype(bf16)
        wkT = Wk[c * HD:(c + 1) * HD, :].T.reshape(H // 128, 128, HD)
        m["wkP"] = np.ascontiguousarray(wkT.transpose(1, 0, 2)).astype(bf16)
        wvT = Wv[c * HD:(c + 1) * HD, :].T.reshape(H // 128, 128, HD)
        m["wvP"] = np.ascontiguousarray(wvT.transpose(1, 0, 2)).astype(bf16)
        m["bqP"] = np.ascontiguousarray(bq[c * QO:(c + 1) * QO].reshape(QH, 128).T)
        m["bkP"] = bk[c * HD:(c + 1) * HD].reshape(128, 1).copy()
        m["bvP"] = bv[c * HD:(c + 1) * HD].reshape(128, 1).copy()
        in_maps.append(m)
    return in_maps


def _gather(results):
    # core c's out row (g*128 + r) holds global token g*1024 + c*128 + r
    stacked = np.stack([results[c]["out"] for c in range(NCORES)])
    full = stacked.reshape(NCORES, NG, 128, H).transpose(1, 0, 2, 3)
    return np.ascontiguousarray(full).reshape(B, S, H)


def kernel(hidden_states, position_ids, Wq, bq, Wk, bk, Wv, bv, Wo, bo):
    global LAST_EXEC_NS, LAST_RESULT
    from concourse.bass_utils import run_bass_kernel_spmd

    if "nc" not in _compiled:
        _compiled["nc"] = _build()
    nc = _compiled["nc"]

    in_maps = _host_inputs(hidden_states, position_ids,
                           Wq, bq, Wk, bk, Wv, bv, Wo, bo)
    trace = os.environ.get("KERNEL_TRACE", "0") == "1"
    res = run_bass_kernel_spmd(nc, in_maps, core_ids=list(range(NCORES)),
                               trace=trace)
    LAST_EXEC_NS = res.exec_time_ns
    LAST_RESULT = res
    return _gather(res.results)


# revision 6
# speedup vs baseline: 1.0640x; 1.0280x over previous
"""Trainium2 Bass kernel for Llama-style GQA attention (B=2,S=2048,H=4096,NH=32,NKV=8,HD=128).

Sharding: tensor-parallel over heads for QKV+attention (core c owns Q-heads
4c..4c+3 and GQA KV-head c), then an AllToAll of the raw attention outputs
(4 x 1MB bf16) converts head-sharding -> token-sharding, and each core runs
the output projection for its own 512 tokens against the full Wo (streamed).
This replaces the old 8 x 8MB fp32 ReduceScatter of O-proj partials.
kernel(**inputs) takes full inputs, returns the full output.
"""

import math
import os
from contextlib import ExitStack

import numpy as np

B, S, H = 2, 2048, 4096
NH, NKV, HD = 32, 8, 128
THETA = 1000000.0
NCORES = 8
QH = NH // NCORES            # 4 q-heads per core
TOK = B * S                  # 4096 tokens (flattened batch*seq)
QO = QH * HD                 # 512 q-out dims per core
TT = TOK // 128              # 32 token tiles of 128
TS = TOK // 512              # 8 token slices of 512
SB = S // 512                # 4 q-slices of 512 per batch
KTB = S // 128               # 16 k-tiles of 128 per batch
NG = TS // 2                 # 4 A2A groups of 1024 tokens
NF = 16                      # O-proj out-dim blocks of 256
FO = H // NF                 # 256

LAST_EXEC_NS = None
LAST_RESULT = None

_compiled = {}


def _build():
    import concourse.bass as bass
    import concourse.mybir as mybir
    import concourse.tile as tile
    from concourse import bacc

    f32 = mybir.dt.float32
    f32r = mybir.dt.float32r            # fp32 w/ 11-bit mantissa: 1 PE cyc/row
    bf = mybir.dt.bfloat16
    nc = bacc.Bacc("TRN2", target_bir_lowering=False, debug=False,
                   num_devices=NCORES)

    def inp(name, shape, dt=f32):
        return nc.dram_tensor(name, shape, dt, kind="ExternalInput").ap()

    # hidden transposed and host-pre-tiled: xTt[ti] is a contiguous
    # (H, 512) block for token slice ti -> single-burst DMA tiles
    xTt = inp("xTt", (TS, H, 512), bf)
    # weight shards host-permuted to SBUF-resident layout [p, tile, out]
    wqP = inp("wqP", (128, H // 128, QO), bf)
    wkP = inp("wkP", (128, H // 128, HD), bf)
    wvP = inp("wvP", (128, H // 128, HD), bf)
    # full Wo, tiled for streaming: woF[f, p, c, o] = Wo[f*FO+o, c*128+p]
    # ([p, c, o] inner order matches the SBUF tile's flatten order)
    woF = inp("woF", (NF, 128, NH, FO), bf)
    bqP = inp("bqP", (128, QH))         # bq shard as [d, head]
    bkP = inp("bkP", (128, 1))
    bvP = inp("bvP", (128, 1))
    boB = inp("boB", (1, H), bf)        # full bo
    cosT = inp("cosT", (HD, TOK), bf)
    sinT = inp("sinT", (HD, TOK), bf)
    rotM = inp("rotM", (HD, HD), bf)  # lhsT for rotate_half_interleaved
    ident = inp("ident", (128, 128), f32r)
    onesr = inp("onesr", (128, 1), f32r)
    ones = inp("ones", (128, 128), bf)
    mask128 = inp("mask128", (128, 128), bf)  # strict-diagonal causal triangle

    out = nc.dram_tensor("out", (TOK // NCORES, H), f32, kind="ExternalOutput").ap()
    # A2A buffers: one pair per 1024-token group so each collective's
    # dependencies stay scoped to its group (comms overlap compute).
    # Layout [dest core j, d partition, head, token] so SBUF<->DRAM DMAs are
    # contiguous 1KB lines per partition.
    a2a_ins = [nc.dram_tensor(f"a2a_in{g}", (NCORES, 128, QH, 128), bf,
                              kind="Internal").ap() for g in range(NG)]
    a2a_outs = [nc.dram_tensor(f"a2a_out{g}", (NCORES, 128, QH, 128), bf,
                               kind="Internal").ap() for g in range(NG)]

    inv_sqrt_hd = 1.0 / math.sqrt(HD)

    def mm(out, lhsT, rhs, **kw):
        nc.tensor.matmul(out, lhsT, rhs, **kw)

    with tile.TileContext(nc) as tc, ExitStack() as stk:
        # ---------------- constants + persistent activations ----------------
        cpool = stk.enter_context(tc.tile_pool(name="consts", bufs=1))
        apool = stk.enter_context(tc.tile_pool(name="acts", bufs=1))

        # resident QKV weight shards: [d-in partition, h tile, out], loaded in
        # 4 interleaved chunks of 8 h-tiles so slice-0 matmuls start after
        # ~1.5MB of DMA instead of 6MB
        wq_res = [apool.tile([128, 8, QO], bf, name=f"wq_res{i}") for i in range(4)]
        wk_res = [apool.tile([128, 8, HD], bf, name=f"wk_res{i}") for i in range(4)]
        wv_res = [apool.tile([128, 8, HD], bf, name=f"wv_res{i}") for i in range(4)]
        for c4 in range(4):
            nc.sync.dma_start(wq_res[c4][:], wqP[:, 8 * c4:8 * (c4 + 1), :])
            nc.sync.dma_start(wk_res[c4][:], wkP[:, 8 * c4:8 * (c4 + 1), :])
            nc.sync.dma_start(wv_res[c4][:], wvP[:, 8 * c4:8 * (c4 + 1), :])

        cos_sb = cpool.tile([128, TOK], bf)
        nc.sync.dma_start(cos_sb[:], cosT[:])
        sin_sb = cpool.tile([128, TOK], bf)
        nc.sync.dma_start(sin_sb[:], sinT[:])
        rot_sb = cpool.tile([128, 128], bf)
        nc.sync.dma_start(rot_sb[:], rotM[:])
        id_sb = cpool.tile([128, 128], f32r)
        nc.sync.dma_start(id_sb[:], ident[:])
        onesr_sb = cpool.tile([128, 1], f32r)
        nc.sync.dma_start(onesr_sb[:], onesr[:])
        ones_sb = cpool.tile([128, 128], bf)
        nc.sync.dma_start(ones_sb[:], ones[:])
        bq_sb = cpool.tile([128, QH], f32)
        nc.sync.dma_start(bq_sb[:], bqP[:])
        bk_sb = cpool.tile([128, 1], f32)
        nc.sync.dma_start(bk_sb[:], bkP[:])
        bv_sb = cpool.tile([128, 1], f32)
        nc.sync.dma_start(bv_sb[:], bvP[:])
        bo_sb = cpool.tile([1, H], bf)
        nc.sync.dma_start(bo_sb[:], boB[:])
        # bo broadcast to all partitions, used in the O-proj PSUM drain
        bo_bc = cpool.tile([128, H], bf)
        nc.gpsimd.partition_broadcast(bo_bc[:], bo_sb[:])

        mask_sb = cpool.tile([128, 128], bf)
        nc.sync.dma_start(mask_sb[:], mask128[:])

        KT = apool.tile([128, TOK], bf)        # K^T (rope'd), grows causally
        Vsb = apool.tile([128, TT, 128], bf)   # V in [t mod 128, t tile, d]
        # post-A2A attention outputs: [d partition, group, src core, head, tok]
        oin = apool.tile([128, NG, NCORES, QH, 128], bf)

        sp = stk.enter_context(tc.tile_pool(name="streams", bufs=4))
        tp = stk.enter_context(tc.tile_pool(name="tmps", bufs=2))
        acp = stk.enter_context(tc.tile_pool(name="accs", bufs=2))
        qtp = stk.enter_context(tc.tile_pool(name="qts", bufs=2))
        vtp = stk.enter_context(tc.tile_pool(name="vts", bufs=2))
        atp = stk.enter_context(tc.tile_pool(name="attw", bufs=4))
        smp = stk.enter_context(tc.tile_pool(name="smalls", bufs=2))
        anp = stk.enter_context(tc.tile_pool(name="atn", bufs=2))
        wop = stk.enter_context(tc.tile_pool(name="wo", bufs=2))
        stp = stk.enter_context(tc.tile_pool(name="ostage", bufs=2))
        pp = stk.enter_context(tc.tile_pool(name="ps", bufs=8, space="PSUM"))

        def ps_tile(shape=(128, 512)):
            return pp.tile(list(shape), f32, name="ps", tag="ps")

        # prefetch the first O-proj weight block during the slice loop
        wo_first = wop.tile([128, NH, FO], bf, name="wo_sb")
        nc.sync.dma_start(wo_first[:], woF[0, :, :, :])

        for ti in range(TS):
            b, j = ti // SB, ti % SB
            t0 = ti * 512
            # ---- QKV projection for this token slice (accumulate over h) ----
            psq = [ps_tile() for _ in range(QH)]
            psk = ps_tile()
            psv = ps_tile()
            for hi in range(H // 128):
                h0 = hi * 128
                c4, hc = hi // 8, hi % 8
                xt = sp.tile([128, 512], bf, name="xt")
                # all xt loads ride the scalar HWDGE queue ALONE: no
                # collective-waiting dma can ever block the activation stream
                nc.scalar.dma_start(xt[:], xTt[ti, h0:h0 + 128, :])
                st = (hi == 0)
                en = (hi == H // 128 - 1)
                for q in range(QH):
                    mm(psq[q][:], wq_res[c4][:, hc, q * 128:(q + 1) * 128],
                       xt[:], start=st, stop=en)
                mm(psk[:], wk_res[c4][:, hc, :], xt[:], start=st, stop=en)
                mm(psv[:], wv_res[c4][:, hc, :], xt[:], start=st, stop=en)

            # bias add (per-partition) while draining PSUM
            QTs = qtp.tile([128, QH, 512], bf, name="QTs")
            VTs = vtp.tile([128, 512], f32r, name="VTs")
            for q in range(QH):
                nc.scalar.add(QTs[:, q, :], psq[q][:], bq_sb[:, q:q + 1])
            nc.scalar.add(KT[:, t0:t0 + 512], psk[:], bk_sb[:, 0:1])
            nc.scalar.add(VTs[:], psv[:], bv_sb[:, 0:1])

            # rope in place on QT / KT slices
            def rope(ap_slice):
                rps = ps_tile()
                mm(rps[:], rot_sb[:], ap_slice, start=True, stop=True)
                t1 = tp.tile([128, 512], bf, name="t1")
                nc.vector.tensor_mul(t1[:], ap_slice, cos_sb[:, t0:t0 + 512])
                t2 = tp.tile([128, 512], bf, name="t2")
                nc.vector.tensor_mul(t2[:], rps[:], sin_sb[:, t0:t0 + 512])
                nc.vector.tensor_add(ap_slice, t1[:], t2[:])

            rope(KT[:, t0:t0 + 512])   # first: scores need K before all Q heads
            for q in range(QH):
                rope(QTs[:, q, :])

            # V^T -> V (PE transpose of 128x128 blocks)
            for s4 in range(4):
                g4 = ti * 4 + s4
                vps = pp.tile([128, 128], f32r, name="vps", tag="ps")
                nc.tensor.transpose(vps[:], VTs[:, s4 * 128:(s4 + 1) * 128],
                                    id_sb[:])
                nc.scalar.copy(Vsb[:, g4, :], vps[:])

            # ---- causal attention for this q slice ----
            # Diagonal k-tiles (ki = 4j + a) only attend to q >= 128a: shrink
            # the matmul free range instead of masking the whole tile; only
            # the strict-diagonal 128-col strip needs the triangle mask.
            nk = 4 * j + 4                # k tiles of 128 within batch b
            ATn = anp.tile([128, QH, 512], bf, name="ATn")
            pending_tail = None           # previous head's softmax epilogue
            for h in range(QH):
                at_ps = ps_tile()
                # exp-sums accumulate on DVE (acc), PE does one final
                # ones-matmul per head instead of one per k-tile
                acc = acp.tile([128, 512], f32r, name="acc")
                a_sbs = {}

                def emit_sc(ki):
                    kg = b * KTB + ki
                    a = ki - 4 * j
                    off = 128 * a if a >= 0 else 0
                    sc_ps = ps_tile()
                    mm(sc_ps[:, off:], KT[:, kg * 128:(kg + 1) * 128],
                       QTs[:, h, off:], start=True, stop=True)
                    a_sb = atp.tile([128, 512], bf, name="a_sb")
                    nc.scalar.activation(a_sb[:, off:], sc_ps[:, off:],
                                         mybir.ActivationFunctionType.Exp,
                                         scale=inv_sqrt_hd)
                    if a >= 0:
                        nc.vector.tensor_mul(a_sb[:, off:off + 128],
                                             a_sb[:, off:off + 128],
                                             mask_sb[:])
                    a_sbs[ki] = (a_sb, off)

                # 2-deep software pipeline: scores/exp run ahead of the AV
                # matmuls so the PE never waits on the ACT/DVE chain
                emit_sc(0)
                if nk > 1:
                    emit_sc(1)
                # previous head's epilogue goes here, after this head's first
                # two score matmuls, so its dn matmul doesn't stall the PE
                # waiting for the DVE exp-sum chain to drain
                if pending_tail is not None:
                    pending_tail()
                for ki in range(nk):
                    if ki + 2 < nk:
                        emit_sc(ki + 2)
                    a_sb, off = a_sbs.pop(ki)
                    kg = b * KTB + ki
                    mm(at_ps[:, off:], Vsb[:, kg, :], a_sb[:, off:],
                       start=(ki == 0), stop=(ki == nk - 1))
                    if ki == 0:
                        nc.scalar.copy(acc[:], a_sb[:])
                    else:
                        nc.vector.tensor_add(acc[:, off:], acc[:, off:],
                                             a_sb[:, off:])

                def make_tail(h=h, at_ps=at_ps, acc=acc):
                    def tail():
                        dn_ps = ps_tile((1, 512))
                        mm(dn_ps[:], onesr_sb[:, 0:1], acc[:],
                           start=True, stop=True)
                        dr = smp.tile([1, 512], f32, name="dr")
                        nc.vector.reciprocal(dr[:], dn_ps[:])
                        rb = smp.tile([128, 512], f32, name="rb")
                        nc.gpsimd.partition_broadcast(rb[:], dr[:])
                        nc.vector.tensor_mul(ATn[:, h, :], at_ps[:], rb[:])
                    return tail

                pending_tail = make_tail()
            pending_tail()

            # ---- stage this slice's attention output for the A2A ----
            g, half = ti // 2, ti % 2
            for jj in range(4):
                nc.sync.dma_start(
                    a2a_ins[g][4 * half + jj, :, :, :],
                    ATn[:, :, jj * 128:(jj + 1) * 128])
            if half == 1:
                # group complete: exchange head-shards for token-shards
                nc.gpsimd.collective_compute(
                    "AllToAll", mybir.AluOpType.bypass,
                    replica_groups=[list(range(NCORES))],
                    ins=[a2a_ins[g].opt()], outs=[a2a_outs[g].opt()],
                )

        # oin loads happen here, NOT inside the slice loop: an HWDGE dma that
        # waits on a collective blocks the issuing engine's whole queue, which
        # stalled later slices' loads when these were interleaved. Groups 0-2
        # finished their A2As long ago (sync queue); group 3's wait parks on
        # the scalar queue, which has nothing left to issue.
        for g in range(NG - 1):
            for s in range(NCORES):
                nc.sync.dma_start(oin[:, g, s, :, :],
                                  a2a_outs[g][s, :, :, :])
        for s in range(NCORES):
            nc.scalar.dma_start(oin[:, NG - 1, s, :, :],
                                a2a_outs[NG - 1][s, :, :, :])

        # ---- O-projection for this core's 512 tokens, full Wo streamed ----
        for f in range(NF):
            f0 = f * FO
            if f == 0:
                wo_sb = wo_first
            else:
                wo_sb = wop.tile([128, NH, FO], bf, name="wo_sb")
                nc.sync.dma_start(wo_sb[:], woF[f, :, :, :])
            for g in range(NG):
                op_ps = ps_tile((128, FO))
                for s in range(NCORES):
                    for hh in range(QH):
                        c = s * QH + hh
                        mm(op_ps[:], oin[:, g, s, hh, :], wo_sb[:, c, :],
                           start=(c == 0), stop=(c == NH - 1))
                st_t = stp.tile([128, FO], f32, name="st_t")
                nc.vector.tensor_add(st_t[:], op_ps[:], bo_bc[:, f0:f0 + FO])
                nc.sync.dma_start(out[g * 128:(g + 1) * 128, f0:f0 + FO],
                                  st_t[:])

    nc.compile()
    return nc


def _host_inputs(hidden_states, position_ids, Wq, bq, Wk, bk, Wv, bv, Wo, bo):
    import ml_dtypes
    bf16 = ml_dtypes.bfloat16
    f = np.float32
    X = np.asarray(hidden_states, f).reshape(TOK, H)
    xT = np.ascontiguousarray(X.T).astype(bf16)
    xTt = np.ascontiguousarray(xT.reshape(H, TS, 512).transpose(1, 0, 2))

    pos = np.asarray(position_ids).astype(f).reshape(TOK)
    inv_freq = (1.0 / (THETA ** (np.arange(0, HD, 2, dtype=f) / HD))).astype(f)
    M = inv_freq[:, None] * pos[None, :]              # [64, TOK]
    cosT = np.repeat(np.cos(M), 2, axis=0).astype(f)  # [128, TOK]
    sinT = np.repeat(np.sin(M), 2, axis=0).astype(f)

    rotM = np.zeros((HD, HD), f)
    for i in range(HD // 2):
        rotM[2 * i + 1, 2 * i] = -1.0   # out[2i]   = -in[2i+1]
        rotM[2 * i, 2 * i + 1] = 1.0    # out[2i+1] =  in[2i]

    Wq, Wk, Wv, Wo = (np.asarray(a, f) for a in (Wq, Wk, Wv, Wo))
    bq, bk, bv, bo = (np.asarray(a, f) for a in (bq, bk, bv, bo))

    # full Wo tiled for streaming: woF[f, p, c, o] = Wo[f*FO+o, c*128+p]
    woF = np.ascontiguousarray(
        Wo.T.reshape(NH, 128, NF, FO).transpose(2, 1, 0, 3)).astype(bf16)

    shared = {
        "xTt": xTt, "cosT": cosT.astype(bf16), "sinT": sinT.astype(bf16),
        "rotM": rotM.astype(bf16),
        "ident": np.eye(128, dtype=f), "ones": np.ones((128, 128), bf16),
        "onesr": np.ones((128, 1), f),
        "woF": woF,
        "boB": np.asarray(bo, f).reshape(1, H).astype(bf16),
        "mask128": (np.arange(128)[None, :]
                    - np.arange(128)[:, None] >= 0).astype(bf16),
    }
    in_maps = []
    for c in range(NCORES):
        m = dict(shared)
        # [p, h-tile, o] resident layout: wT[h, o] with h = ht*128 + p
        wqT = Wq[c * QO:(c + 1) * QO, :].T.reshape(H // 128, 128, QO)
        m["wqP"] = np.ascontiguousarray(wqT.transpose(1, 0, 2)).astype(bf16)
        wkT = Wk[c * HD:(c + 1) * HD, :].T.reshape(H // 128, 128, HD)
        m["wkP"] = np.ascontiguousarray(wkT.transpose(1, 0, 2)).astype(bf16)
        wvT = Wv[c * HD:(c + 1) * HD, :].T.reshape(H // 128, 128, HD)
        m["wvP"] = np.ascontiguousarray(wvT.transpose(1, 0, 2)).astype(bf16)
        m["bqP"] = np.ascontiguousarray(bq[c * QO:(c + 1) * QO].reshape(QH, 128).T)
        m["bkP"] = bk[c * HD:(c + 1) * HD].reshape(128, 1).copy()
        m["bvP"] = bv[c * HD:(c + 1) * HD].reshape(128, 1).copy()
        in_maps.append(m)
    return in_maps


def _gather(results):
    # core c's out row (g*128 + r) holds global token g*1024 + c*128 + r
    stacked = np.stack([results[c]["out"] for c in range(NCORES)])
    full = stacked.reshape(NCORES, NG, 128, H).transpose(1, 0, 2, 3)
    return np.ascontiguousarray(full).reshape(B, S, H)


def kernel(hidden_states, position_ids, Wq, bq, Wk, bk, Wv, bv, Wo, bo):
    global LAST_EXEC_NS, LAST_RESULT
    from concourse.bass_utils import run_bass_kernel_spmd

    if "nc" not in _compiled:
        _compiled["nc"] = _build()
    nc = _compiled["nc"]

    in_maps = _host_inputs(hidden_states, position_ids,
                           Wq, bq, Wk, bk, Wv, bv, Wo, bo)
    trace = os.environ.get("KERNEL_TRACE", "0") == "1"
    res = run_bass_kernel_spmd(nc, in_maps, core_ids=list(range(NCORES)),
                               trace=trace)
    LAST_EXEC_NS = res.exec_time_ns
    LAST_RESULT = res
    return _gather(res.results)


# revision 7
# speedup vs baseline: 1.0656x; 1.0016x over previous
"""Trainium2 Bass kernel for Llama-style GQA attention (B=2,S=2048,H=4096,NH=32,NKV=8,HD=128).

Sharding: tensor-parallel over heads for QKV+attention (core c owns Q-heads
4c..4c+3 and GQA KV-head c), then an AllToAll of the raw attention outputs
(4 x 1MB bf16) converts head-sharding -> token-sharding, and each core runs
the output projection for its own 512 tokens against the full Wo (streamed).
This replaces the old 8 x 8MB fp32 ReduceScatter of O-proj partials.
kernel(**inputs) takes full inputs, returns the full output.
"""

import math
import os
from contextlib import ExitStack

import numpy as np

B, S, H = 2, 2048, 4096
NH, NKV, HD = 32, 8, 128
THETA = 1000000.0
NCORES = 8
QH = NH // NCORES            # 4 q-heads per core
TOK = B * S                  # 4096 tokens (flattened batch*seq)
QO = QH * HD                 # 512 q-out dims per core
TT = TOK // 128              # 32 token tiles of 128
TS = TOK // 512              # 8 token slices of 512
SB = S // 512                # 4 q-slices of 512 per batch
KTB = S // 128               # 16 k-tiles of 128 per batch
NG = TS // 2                 # 4 A2A groups of 1024 tokens
NF = 16                      # O-proj out-dim blocks of 256
FO = H // NF                 # 256

LAST_EXEC_NS = None
LAST_RESULT = None

_compiled = {}


def _build():
    import concourse.bass as bass
    import concourse.mybir as mybir
    import concourse.tile as tile
    from concourse import bacc

    f32 = mybir.dt.float32
    f32r = mybir.dt.float32r            # fp32 w/ 11-bit mantissa: 1 PE cyc/row
    bf = mybir.dt.bfloat16
    nc = bacc.Bacc("TRN2", target_bir_lowering=False, debug=False,
                   num_devices=NCORES)

    def inp(name, shape, dt=f32):
        return nc.dram_tensor(name, shape, dt, kind="ExternalInput").ap()

    # hidden transposed and host-pre-tiled: xTt[ti] is a contiguous
    # (H, 512) block for token slice ti -> single-burst DMA tiles
    xTt = inp("xTt", (TS, H, 512), bf)
    # weight shards host-permuted to SBUF-resident layout [p, tile, out]
    wqP = inp("wqP", (128, H // 128, QO), bf)
    wkP = inp("wkP", (128, H // 128, HD), bf)
    wvP = inp("wvP", (128, H // 128, HD), bf)
    # full Wo, tiled for streaming: woF[f, p, c, o] = Wo[f*FO+o, c*128+p]
    # ([p, c, o] inner order matches the SBUF tile's flatten order)
    woF = inp("woF", (NF, 128, NH, FO), bf)
    bqP = inp("bqP", (128, QH))         # bq shard as [d, head]
    bkP = inp("bkP", (128, 1))
    bvP = inp("bvP", (128, 1))
    boB = inp("boB", (1, H), bf)        # full bo
    cosT = inp("cosT", (HD, TOK), bf)
    sinT = inp("sinT", (HD, TOK), bf)
    rotM = inp("rotM", (HD, HD), bf)  # lhsT for rotate_half_interleaved
    ident = inp("ident", (128, 128), f32r)
    onesr = inp("onesr", (128, 1), f32r)
    ones = inp("ones", (128, 128), bf)
    mask128 = inp("mask128", (128, 128), bf)  # strict-diagonal causal triangle

    out = nc.dram_tensor("out", (TOK // NCORES, H), f32, kind="ExternalOutput").ap()
    # A2A buffers: one pair per 1024-token group so each collective's
    # dependencies stay scoped to its group (comms overlap compute).
    # Layout [dest core j, d partition, head, token] so SBUF<->DRAM DMAs are
    # contiguous 1KB lines per partition.
    a2a_ins = [nc.dram_tensor(f"a2a_in{g}", (NCORES, 128, QH, 128), bf,
                              kind="Internal").ap() for g in range(NG)]
    a2a_outs = [nc.dram_tensor(f"a2a_out{g}", (NCORES, 128, QH, 128), bf,
                               kind="Internal").ap() for g in range(NG)]

    inv_sqrt_hd = 1.0 / math.sqrt(HD)

    def mm(out, lhsT, rhs, **kw):
        nc.tensor.matmul(out, lhsT, rhs, **kw)

    with tile.TileContext(nc) as tc, ExitStack() as stk:
        # ---------------- constants + persistent activations ----------------
        cpool = stk.enter_context(tc.tile_pool(name="consts", bufs=1))
        apool = stk.enter_context(tc.tile_pool(name="acts", bufs=1))

        # resident QKV weight shards: [d-in partition, h tile, out], loaded in
        # 4 interleaved chunks of 8 h-tiles so slice-0 matmuls start after
        # ~1.5MB of DMA instead of 6MB
        wq_res = [apool.tile([128, 8, QO], bf, name=f"wq_res{i}") for i in range(4)]
        wk_res = [apool.tile([128, 8, HD], bf, name=f"wk_res{i}") for i in range(4)]
        wv_res = [apool.tile([128, 8, HD], bf, name=f"wv_res{i}") for i in range(4)]
        for c4 in range(4):
            nc.sync.dma_start(wq_res[c4][:], wqP[:, 8 * c4:8 * (c4 + 1), :])
            nc.sync.dma_start(wk_res[c4][:], wkP[:, 8 * c4:8 * (c4 + 1), :])
            nc.sync.dma_start(wv_res[c4][:], wvP[:, 8 * c4:8 * (c4 + 1), :])

        cos_sb = cpool.tile([128, TOK], bf)
        nc.sync.dma_start(cos_sb[:], cosT[:])
        sin_sb = cpool.tile([128, TOK], bf)
        nc.sync.dma_start(sin_sb[:], sinT[:])
        rot_sb = cpool.tile([128, 128], bf)
        nc.sync.dma_start(rot_sb[:], rotM[:])
        id_sb = cpool.tile([128, 128], f32r)
        nc.sync.dma_start(id_sb[:], ident[:])
        onesr_sb = cpool.tile([128, 1], f32r)
        nc.sync.dma_start(onesr_sb[:], onesr[:])
        ones_sb = cpool.tile([128, 128], bf)
        nc.sync.dma_start(ones_sb[:], ones[:])
        bq_sb = cpool.tile([128, QH], f32)
        nc.sync.dma_start(bq_sb[:], bqP[:])
        bk_sb = cpool.tile([128, 1], f32)
        nc.sync.dma_start(bk_sb[:], bkP[:])
        bv_sb = cpool.tile([128, 1], f32)
        nc.sync.dma_start(bv_sb[:], bvP[:])
        bo_sb = cpool.tile([1, H], bf)
        nc.sync.dma_start(bo_sb[:], boB[:])
        # bo broadcast to all partitions, used in the O-proj PSUM drain
        bo_bc = cpool.tile([128, H], bf)
        nc.gpsimd.partition_broadcast(bo_bc[:], bo_sb[:])

        mask_sb = cpool.tile([128, 128], bf)
        nc.sync.dma_start(mask_sb[:], mask128[:])

        KT = apool.tile([128, TOK], bf)        # K^T (rope'd), grows causally
        Vsb = apool.tile([128, TT, 128], bf)   # V in [t mod 128, t tile, d]
        # post-A2A attention outputs: [d partition, group, src core, head, tok]
        oin = apool.tile([128, NG, NCORES, QH, 128], bf)

        sp = stk.enter_context(tc.tile_pool(name="streams", bufs=4))
        tp = stk.enter_context(tc.tile_pool(name="tmps", bufs=2))
        acp = stk.enter_context(tc.tile_pool(name="accs", bufs=2))
        qtp = stk.enter_context(tc.tile_pool(name="qts", bufs=2))
        vtp = stk.enter_context(tc.tile_pool(name="vts", bufs=2))
        atp = stk.enter_context(tc.tile_pool(name="attw", bufs=4))
        smp = stk.enter_context(tc.tile_pool(name="smalls", bufs=2))
        anp = stk.enter_context(tc.tile_pool(name="atn", bufs=2))
        wop = stk.enter_context(tc.tile_pool(name="wo", bufs=2))
        stp = stk.enter_context(tc.tile_pool(name="ostage", bufs=2))
        pp = stk.enter_context(tc.tile_pool(name="ps", bufs=8, space="PSUM"))

        def ps_tile(shape=(128, 512)):
            return pp.tile(list(shape), f32, name="ps", tag="ps")

        # prefetch the first O-proj weight block during the slice loop
        wo_first = wop.tile([128, NH, FO], bf, name="wo_sb")
        nc.sync.dma_start(wo_first[:], woF[0, :, :, :])

        for ti in range(TS):
            b, j = ti // SB, ti % SB
            t0 = ti * 512
            # ---- QKV projection for this token slice (accumulate over h) ----
            psq = [ps_tile() for _ in range(QH)]
            psk = ps_tile()
            psv = ps_tile()
            for hi in range(H // 128):
                h0 = hi * 128
                c4, hc = hi // 8, hi % 8
                xt = sp.tile([128, 512], bf, name="xt")
                # slice 0 on the scalar queue (overlaps the weight DMAs on
                # sync at startup); the rest on sync, which carries no
                # collective-waiting dma that could stall the stream
                dq = nc.scalar if ti == 0 else nc.sync
                dq.dma_start(xt[:], xTt[ti, h0:h0 + 128, :])
                st = (hi == 0)
                en = (hi == H // 128 - 1)
                for q in range(QH):
                    mm(psq[q][:], wq_res[c4][:, hc, q * 128:(q + 1) * 128],
                       xt[:], start=st, stop=en)
                mm(psk[:], wk_res[c4][:, hc, :], xt[:], start=st, stop=en)
                mm(psv[:], wv_res[c4][:, hc, :], xt[:], start=st, stop=en)

            # bias add (per-partition) while draining PSUM
            QTs = qtp.tile([128, QH, 512], bf, name="QTs")
            VTs = vtp.tile([128, 512], f32r, name="VTs")
            for q in range(QH):
                nc.scalar.add(QTs[:, q, :], psq[q][:], bq_sb[:, q:q + 1])
            nc.scalar.add(KT[:, t0:t0 + 512], psk[:], bk_sb[:, 0:1])
            nc.scalar.add(VTs[:], psv[:], bv_sb[:, 0:1])

            # rope in place on QT / KT slices
            def rope(ap_slice):
                rps = ps_tile()
                mm(rps[:], rot_sb[:], ap_slice, start=True, stop=True)
                t1 = tp.tile([128, 512], bf, name="t1")
                nc.vector.tensor_mul(t1[:], ap_slice, cos_sb[:, t0:t0 + 512])
                t2 = tp.tile([128, 512], bf, name="t2")
                nc.vector.tensor_mul(t2[:], rps[:], sin_sb[:, t0:t0 + 512])
                nc.vector.tensor_add(ap_slice, t1[:], t2[:])

            rope(KT[:, t0:t0 + 512])   # first: scores need K before all Q heads
            for q in range(QH):
                rope(QTs[:, q, :])

            # V^T -> V (PE transpose of 128x128 blocks)
            for s4 in range(4):
                g4 = ti * 4 + s4
                vps = pp.tile([128, 128], f32r, name="vps", tag="ps")
                nc.tensor.transpose(vps[:], VTs[:, s4 * 128:(s4 + 1) * 128],
                                    id_sb[:])
                nc.scalar.copy(Vsb[:, g4, :], vps[:])

            # ---- causal attention for this q slice ----
            # Diagonal k-tiles (ki = 4j + a) only attend to q >= 128a: shrink
            # the matmul free range instead of masking the whole tile; only
            # the strict-diagonal 128-col strip needs the triangle mask.
            nk = 4 * j + 4                # k tiles of 128 within batch b
            ATn = anp.tile([128, QH, 512], bf, name="ATn")
            pending_tail = None           # previous head's softmax epilogue
            for h in range(QH):
                at_ps = ps_tile()
                # exp-sums accumulate on DVE (acc), PE does one final
                # ones-matmul per head instead of one per k-tile
                acc = acp.tile([128, 512], f32r, name="acc")
                a_sbs = {}

                def emit_sc(ki):
                    kg = b * KTB + ki
                    a = ki - 4 * j
                    off = 128 * a if a >= 0 else 0
                    sc_ps = ps_tile()
                    mm(sc_ps[:, off:], KT[:, kg * 128:(kg + 1) * 128],
                       QTs[:, h, off:], start=True, stop=True)
                    a_sb = atp.tile([128, 512], bf, name="a_sb")
                    nc.scalar.activation(a_sb[:, off:], sc_ps[:, off:],
                                         mybir.ActivationFunctionType.Exp,
                                         scale=inv_sqrt_hd)
                    if a >= 0:
                        nc.vector.tensor_mul(a_sb[:, off:off + 128],
                                             a_sb[:, off:off + 128],
                                             mask_sb[:])
                    a_sbs[ki] = (a_sb, off)

                # 2-deep software pipeline: scores/exp run ahead of the AV
                # matmuls so the PE never waits on the ACT/DVE chain
                emit_sc(0)
                if nk > 1:
                    emit_sc(1)
                # previous head's epilogue goes here, after this head's first
                # two score matmuls, so its dn matmul doesn't stall the PE
                # waiting for the DVE exp-sum chain to drain
                if pending_tail is not None:
                    pending_tail()
                for ki in range(nk):
                    if ki + 2 < nk:
                        emit_sc(ki + 2)
                    a_sb, off = a_sbs.pop(ki)
                    kg = b * KTB + ki
                    mm(at_ps[:, off:], Vsb[:, kg, :], a_sb[:, off:],
                       start=(ki == 0), stop=(ki == nk - 1))
                    if ki == 0:
                        nc.scalar.copy(acc[:], a_sb[:])
                    else:
                        nc.vector.tensor_add(acc[:, off:], acc[:, off:],
                                             a_sb[:, off:])

                def make_tail(h=h, at_ps=at_ps, acc=acc):
                    def tail():
                        dn_ps = ps_tile((1, 512))
                        mm(dn_ps[:], onesr_sb[:, 0:1], acc[:],
                           start=True, stop=True)
                        dr = smp.tile([1, 512], f32, name="dr")
                        nc.vector.reciprocal(dr[:], dn_ps[:])
                        rb = smp.tile([128, 512], f32, name="rb")
                        nc.gpsimd.partition_broadcast(rb[:], dr[:])
                        nc.vector.tensor_mul(ATn[:, h, :], at_ps[:], rb[:])
                    return tail

                pending_tail = make_tail()
            pending_tail()

            # ---- stage this slice's attention output for the A2A ----
            g, half = ti // 2, ti % 2
            for jj in range(4):
                nc.scalar.dma_start(
                    a2a_ins[g][4 * half + jj, :, :, :],
                    ATn[:, :, jj * 128:(jj + 1) * 128])
            if half == 1:
                # group complete: exchange head-shards for token-shards
                nc.gpsimd.collective_compute(
                    "AllToAll", mybir.AluOpType.bypass,
                    replica_groups=[list(range(NCORES))],
                    ins=[a2a_ins[g].opt()], outs=[a2a_outs[g].opt()],
                )

        # all oin loads ride the scalar queue, pinned to the END of its
        # stream via tile_wait_until so the scheduler cannot hoist them into
        # the slice loop (an HWDGE dma waiting on a collective blocks every
        # instruction behind it on the issuing engine)
        for g in range(NG):
            with tc.tile_wait_until(50 + g):
                for s in range(NCORES):
                    nc.scalar.dma_start(oin[:, g, s, :, :],
                                        a2a_outs[g][s, :, :, :])

        # ---- O-projection for this core's 512 tokens, full Wo streamed ----
        for f in range(NF):
            f0 = f * FO
            if f == 0:
                wo_sb = wo_first
            else:
                wo_sb = wop.tile([128, NH, FO], bf, name="wo_sb")
                nc.sync.dma_start(wo_sb[:], woF[f, :, :, :])
            for g in range(NG):
                op_ps = ps_tile((128, FO))
                for s in range(NCORES):
                    for hh in range(QH):
                        c = s * QH + hh
                        mm(op_ps[:], oin[:, g, s, hh, :], wo_sb[:, c, :],
                           start=(c == 0), stop=(c == NH - 1))
                st_t = stp.tile([128, FO], f32, name="st_t")
                nc.vector.tensor_add(st_t[:], op_ps[:], bo_bc[:, f0:f0 + FO])
                nc.sync.dma_start(out[g * 128:(g + 1) * 128, f0:f0 + FO],
                                  st_t[:])

    nc.compile()
    return nc


def _host_inputs(hidden_states, position_ids, Wq, bq, Wk, bk, Wv, bv, Wo, bo):
    import ml_dtypes
    bf16 = ml_dtypes.bfloat16
    f = np.float32
    X = np.asarray(hidden_states, f).reshape(TOK, H)
    xT = np.ascontiguousarray(X.T).astype(bf16)
    xTt = np.ascontiguousarray(xT.reshape(H, TS, 512).transpose(1, 0, 2))

    pos = np.asarray(position_ids).astype(f).reshape(TOK)
    inv_freq = (1.0 / (THETA ** (np.arange(0, HD, 2, dtype=f) / HD))).astype(f)
    M = inv_freq[:, None] * pos[None, :]              # [64, TOK]
    cosT = np.repeat(np.cos(M), 2, axis=0).astype(f)  # [128, TOK]
    sinT = np.repeat(np.sin(M), 2, axis=0).astype(f)

    rotM = np.zeros((HD, HD), f)
    for i in range(HD // 2):
        rotM[2 * i + 1, 2 * i] = -1.0   # out[2i]   = -in[2i+1]
        rotM[2 * i, 2 * i + 1] = 1.0    # out[2i+1] =  in[2i]

    Wq, Wk, Wv, Wo = (np.asarray(a, f) for a in (Wq, Wk, Wv, Wo))
    bq, bk, bv, bo = (np.asarray(a, f) for a in (bq, bk, bv, bo))

    # full Wo tiled for streaming: woF[f, p, c, o] = Wo[f*FO+o, c*128+p]
    woF = np.ascontiguousarray(
        Wo.T.reshape(NH, 128, NF, FO).transpose(2, 1, 0, 3)).astype(bf16)

    shared = {
        "xTt": xTt, "cosT": cosT.astype(bf16), "sinT": sinT.astype(bf16),
        "rotM": rotM.astype(bf16),
        "ident": np.eye(128, dtype=f), "ones": np.ones((128, 128), bf16),
        "onesr": np.ones((128, 1), f),
        "woF": woF,
        "boB": np.asarray(bo, f).reshape(1, H).astype(bf16),
        "mask128": (np.arange(128)[None, :]
                    - np.arange(128)[:, None] >= 0).astype(bf16),
    }
    in_maps = []
    for c in range(NCORES):
        m = dict(shared)
        # [p, h-tile, o] resident layout: wT[h, o] with h = ht*128 + p
        wqT = Wq[c * QO:(c + 1) * QO, :].T.reshape(H // 128, 128, QO)
        m["wqP"] = np.ascontiguousarray(wqT.transpose(1, 0, 2)).astype(bf16)
        wkT = Wk[c * HD:(c + 1) * HD, :].T.reshape(H // 128, 128, HD)
        m["wkP"] = np.ascontiguousarray(wkT.transpose(1, 0, 2)).astype(bf16)
        wvT = Wv[c * HD:(c + 1) * HD, :].T.reshape(H // 128, 128, HD)
        m["wvP"] = np.ascontiguousarray(wvT.transpose(1, 0, 2)).astype(bf16)
        m["bqP"] = np.ascontiguousarray(bq[c * QO:(c + 1) * QO].reshape(QH, 128).T)
        m["bkP"] = bk[c * HD:(c + 1) * HD].reshape(128, 1).copy()
        m["bvP"] = bv[c * HD:(c + 1) * HD].reshape(128, 1).copy()
        in_maps.append(m)
    return in_maps


def _gather(results):
    # core c's out row (g*128 + r) holds global token g*1024 + c*128 + r
    stacked = np.stack([results[c]["out"] for c in range(NCORES)])
    full = stacked.reshape(NCORES, NG, 128, H).transpose(1, 0, 2, 3)
    return np.ascontiguousarray(full).reshape(B, S, H)


def kernel(hidden_states, position_ids, Wq, bq, Wk, bk, Wv, bv, Wo, bo):
    global LAST_EXEC_NS, LAST_RESULT
    from concourse.bass_utils import run_bass_kernel_spmd

    if "nc" not in _compiled:
        _compiled["nc"] = _build()
    nc = _compiled["nc"]

    in_maps = _host_inputs(hidden_states, position_ids,
                           Wq, bq, Wk, bk, Wv, bv, Wo, bo)
    trace = os.environ.get("KERNEL_TRACE", "0") == "1"
    res = run_bass_kernel_spmd(nc, in_maps, core_ids=list(range(NCORES)),
                               trace=trace)
    LAST_EXEC_NS = res.exec_time_ns
    LAST_RESULT = res
    return _gather(res.results)
